# revision 9
# baseline (speedup 1.0000x reference)
"""Trainium2 Bass kernel for nn_Attention5 (sparse_attention).

Math (B=32, S=2048, H=256, Z=64):
    k = enc @ Wk ; q = dec @ Wq ; v = lat @ Wv
    A[b,i,j] = q_i . k_j / sqrt(H)
    A masked over QUERY axis i (rows with mask=0 -> -1e9)
    attn = softmax(A, axis=-2)          # over i (queries) for each key j
    out  = attn @ v                     # [B,S,Z]
    returns (out, attn)

Strategy:
  * Data-parallel over batch: 4 batches per core x 8 cores.
  * Reparametrize A = dec @ W2 @ enc.T with W2 = (Wq/sqrt(H)) @ Wk.T computed
    host-side in fp64 - removes the K projection entirely.
  * Device computes attn TRANSPOSED (attnT[j,i]) so the softmax reduction is
    along the free axis.  Host returns a transposed view.
  * Masked queries produce exactly-0 attn rows (exp underflow), identical to
    the reference.  Host packs the unmasked query columns into a fixed
    NPACK block (sized from the actual mask); zero columns are never computed
    or written (outputs are pre-zeroed), then host scatters back.
  * Precision: P = dec@W2 projection and P@enc.T logits use a 3-pass bf16
    split (hi*hi + hi*lo + lo*hi): ~5e-6 rel err at full bf16 matmul speed.
    (fp32 matmul measures 10x slower on TRN2; 1-pass f32r/tf32 is too coarse
    for the exp.)  attn.V runs in f32r (tf32).
"""
import sys, os
sys.path.insert(0, "/opt/trn_rl_repo")
import numpy as np

import concourse.bass as bass
import concourse.bacc as bacc
import concourse.tile as tile
import concourse.mybir as mybir
from concourse.bass_utils import run_bass_kernel_spmd

F32 = mybir.dt.float32
F32R = mybir.dt.float32r
BF16 = mybir.dt.bfloat16
AF = mybir.ActivationFunctionType
AX = mybir.AxisListType
OP = mybir.AluOpType

B, S, H, Z = 32, 2048, 256, 64
NCORES = 8
NB = B // NCORES          # batches per core
HB = H // 128             # 2 h-blocks (contraction)
GB = H // 128             # 2 g-blocks (projected dim of P)
JB = S // 128             # 16 key blocks

_CACHE = {}


def _chunks(width, step=512):
    out, c = [], 0
    while c < width:
        out.append((c, min(step, width - c)))
        c += step
    return out


def _split_pair(nc, src_ap, hi_tile, lo_tile, sub_engine, copy_engine=None):
    """hi = bf16(src); lo = bf16(src - hi). src may be SBUF or PSUM."""
    (copy_engine or nc.scalar).tensor_copy(hi_tile[:], src_ap) \
        if copy_engine is not None else nc.scalar.copy(hi_tile[:], src_ap)
    sub_engine.tensor_tensor(lo_tile[:], src_ap, hi_tile[:], op=OP.subtract)


def build_program(npack):
    nc = bacc.Bacc("TRN2", target_bir_lowering=False, debug=False)
    ICH = _chunks(npack)

    decT = nc.dram_tensor("decT", [NB, H, npack], F32, kind="ExternalInput").ap()
    encT = nc.dram_tensor("encT", [NB, H, S], F32, kind="ExternalInput").ap()
    latT = nc.dram_tensor("latT", [NB, Z, S], F32, kind="ExternalInput").ap()
    w2 = nc.dram_tensor("w2", [H, H], F32, kind="ExternalInput").ap()
    wv = nc.dram_tensor("wv", [Z, Z], F32, kind="ExternalInput").ap()

    attnT = nc.dram_tensor("attnT", [NB, S, npack], F32R, kind="ExternalOutput").ap()
    outT = nc.dram_tensor("outT", [NB, Z, npack], F32, kind="ExternalOutput").ap()

    with tile.TileContext(nc) as tc:
        with (
            tc.tile_pool(name="wpool", bufs=1) as wpool,
            tc.tile_pool(name="io", bufs=1) as io,
            tc.tile_pool(name="split", bufs=2) as split,
            tc.tile_pool(name="qk", bufs=2) as qk,
            tc.tile_pool(name="vpool", bufs=2) as vpool,
            tc.tile_pool(name="epool", bufs=3) as epool,
            tc.tile_pool(name="small", bufs=6) as small,
            tc.tile_pool(name="osb", bufs=2) as osb,
            tc.tile_pool(name="atp", bufs=4, space="PSUM") as atp,
            tc.tile_pool(name="projp", bufs=1, space="PSUM") as projp,
            tc.tile_pool(name="outp", bufs=1, space="PSUM") as outp,
        ):
            # ---- weights: load + bf16 split (once) ----
            w2_b, w2_l = [], []
            for hb in range(HB):
                wf = wpool.tile([128, H], F32, tag=f"w2f{hb}", name=f"w2f{hb}")
                nc.sync.dma_start(out=wf[:], in_=w2[hb * 128:(hb + 1) * 128, :])
                hi = wpool.tile([128, H], BF16, tag=f"w2b{hb}", name=f"w2b{hb}")
                lo = wpool.tile([128, H], BF16, tag=f"w2l{hb}", name=f"w2l{hb}")
                _split_pair(nc, wf[:], hi, lo, nc.vector)
                w2_b.append(hi)
                w2_l.append(lo)
            wvf = wpool.tile([Z, Z], F32, tag="wvf")
            nc.sync.dma_start(out=wvf[:], in_=wv[:])
            wv_b = wpool.tile([Z, Z], BF16, tag="wvb")
            wv_l = wpool.tile([Z, Z], BF16, tag="wvl")
            _split_pair(nc, wvf[:], wv_b, wv_l, nc.vector)

            def load_split(b):
                # ---- load + split activations ----
                dec_b, dec_l, enc_b, enc_l = [], [], [], []
                for hb in range(HB):
                    df = io.tile([128, npack], F32, tag=f"df{hb}", name=f"df{hb}")
                    nc.sync.dma_start(out=df[:], in_=decT[b, hb * 128:(hb + 1) * 128, :])
                    dbt = split.tile([128, npack], BF16, tag=f"db{hb}", name=f"db{hb}")
                    dlt = split.tile([128, npack], BF16, tag=f"dl{hb}", name=f"dl{hb}")
                    _split_pair(nc, df[:], dbt, dlt, nc.gpsimd, copy_engine=nc.gpsimd)
                    dec_b.append(dbt); dec_l.append(dlt)

                    ef = io.tile([128, S], F32, tag=f"ef{hb}", name=f"ef{hb}")
                    nc.sync.dma_start(out=ef[:], in_=encT[b, hb * 128:(hb + 1) * 128, :])
                    ebt = split.tile([128, S], BF16, tag=f"eb{hb}", name=f"eb{hb}")
                    elt = split.tile([128, S], BF16, tag=f"el{hb}", name=f"el{hb}")
                    _split_pair(nc, ef[:], ebt, elt, nc.gpsimd, copy_engine=nc.gpsimd)
                    enc_b.append(ebt); enc_l.append(elt)

                lf = io.tile([Z, S], F32, tag="lf")
                nc.sync.dma_start(out=lf[:], in_=latT[b, :, :])
                lat_b = split.tile([Z, S], BF16, tag="latb")
                lat_l = split.tile([Z, S], BF16, tag="latl")
                _split_pair(nc, lf[:], lat_b, lat_l, nc.gpsimd, copy_engine=nc.gpsimd)

                return dec_b, dec_l, enc_b, enc_l, lat_b, lat_l

            def proj_v(b, ls_state):
                dec_b, dec_l, enc_b, enc_l, lat_b, lat_l = ls_state
                # ---- projection PT[g, i-pack] = W2.T @ decT (3-pass bf16) ----
                p_b, p_l = [], []
                for g in range(GB):
                    pb = qk.tile([128, npack], BF16, tag=f"pb{g}", name=f"pb{g}")
                    pl = qk.tile([128, npack], BF16, tag=f"pl{g}", name=f"pl{g}")
                    p_b.append(pb); p_l.append(pl)
                    for (c0, cw) in ICH:
                        ps = projp.tile([128, 512], F32, tag="pj", name="pproj2")
                        first = True
                        for hb in range(HB):
                            wsl_b = w2_b[hb][:, g * 128:(g + 1) * 128]
                            wsl_l = w2_l[hb][:, g * 128:(g + 1) * 128]
                            nc.tensor.matmul(ps[:, :cw], wsl_b,
                                             dec_b[hb][:, c0:c0 + cw],
                                             start=first, stop=False)
                            first = False
                            nc.tensor.matmul(ps[:, :cw], wsl_b,
                                             dec_l[hb][:, c0:c0 + cw],
                                             start=False, stop=False)
                            nc.tensor.matmul(ps[:, :cw], wsl_l,
                                             dec_b[hb][:, c0:c0 + cw],
                                             start=False, stop=(hb == HB - 1))
                        nc.scalar.copy(pb[:, c0:c0 + cw], ps[:, :cw])
                        nc.vector.tensor_tensor(pl[:, c0:c0 + cw], ps[:, :cw],
                                                pb[:, c0:c0 + cw], op=OP.subtract)

                # ---- V[j, d] (3-pass bf16), rounded to f32r ----
                v_sb = vpool.tile([128, JB * Z], F32R, tag="vsb")
                for jb in range(JB):
                    ps = projp.tile([128, 512], F32, tag="pj", name="pv")
                    lsl_b = lat_b[:, jb * 128:(jb + 1) * 128]
                    lsl_l = lat_l[:, jb * 128:(jb + 1) * 128]
                    nc.tensor.matmul(ps[:, :Z], lsl_b, wv_b[:], start=True, stop=False)
                    nc.tensor.matmul(ps[:, :Z], lsl_b, wv_l[:], start=False, stop=False)
                    nc.tensor.matmul(ps[:, :Z], lsl_l, wv_b[:], start=False, stop=True)
                    nc.scalar.copy(v_sb[:, jb * Z:(jb + 1) * Z], ps[:, :Z])

                return enc_b, enc_l, p_b, p_l, v_sb

            def attention(b, state, mid_hooks):
                enc_b, enc_l, p_b, p_l, v_sb = state
                # ---- attention over key blocks: A^T[j,i] = enc @ P^T ----
                po = outp.tile([Z, npack], F32, tag="po")
                for jb in range(JB):
                    if jb in mid_hooks:
                        mid_hooks[jb]()
                    chunks_ps = [atp.tile([128, 512], F32, tag="at", name=f"pat{ci}")
                                 for ci in range(len(ICH))]
                    # stationary-major: 6 stationaries x all chunks each
                    started = [False] * len(ICH)
                    n_mm = [0] * len(ICH)
                    passes = []
                    for hb in range(HB):
                        esl_b = enc_b[hb][:, jb * 128:(jb + 1) * 128]
                        esl_l = enc_l[hb][:, jb * 128:(jb + 1) * 128]
                        passes += [(esl_b, p_b[hb]), (esl_b, p_l[hb]),
                                   (esl_l, p_b[hb])]
                    for pi, (stat, mov) in enumerate(passes):
                        for ci, (c0, cw) in enumerate(ICH):
                            nc.tensor.matmul(chunks_ps[ci][:, :cw], stat,
                                             mov[:, c0:c0 + cw],
                                             start=(pi == 0),
                                             stop=(pi == len(passes) - 1))

                    # row max (negated) over the packed width
                    nmaxes = []
                    for ci, (c0, cw) in enumerate(ICH):
                        nm_ = small.tile([128, 1], F32, tag="nm", name=f"nm{ci}")
                        nc.vector.tensor_reduce(nm_[:], chunks_ps[ci][:, :cw],
                                                axis=AX.X, op=OP.max, negate=True)
                        nmaxes.append(nm_)
                    nmax = nmaxes[0]
                    for ci in range(1, len(ICH)):
                        nm2 = small.tile([128, 1], F32, tag="nmax", name=f"nmx{ci}")
                        nc.vector.tensor_tensor(nm2[:], nmax[:], nmaxes[ci][:],
                                                op=OP.min)
                        nmax = nm2

                    # exp(A - max) with per-chunk row-sum accumulation
                    exp_sb = epool.tile([128, npack], F32, tag="exp")
                    parts = []
                    for ci, (c0, cw) in enumerate(ICH):
                        pa = small.tile([128, 1], F32, tag="pa", name=f"pa{ci}")
                        nc.scalar.activation(exp_sb[:, c0:c0 + cw],
                                             chunks_ps[ci][:, :cw], AF.Exp,
                                             bias=nmax[:], scale=1.0,
                                             accum_out=pa[:])
                        parts.append(pa)
                    rsum = parts[0]
                    for ci in range(1, len(ICH)):
                        rs2 = small.tile([128, 1], F32, tag="rsum", name=f"rs{ci}")
                        nc.vector.tensor_tensor(rs2[:], rsum[:], parts[ci][:],
                                                op=OP.add)
                        rsum = rs2
                    recip = small.tile([128, 1], F32, tag="recip")
                    nc.vector.reciprocal(recip[:], rsum[:])

                    # normalize -> f32r attn rows; write out; accumulate output
                    attn_r = epool.tile([128, npack], F32R, tag="attnr")
                    nc.vector.tensor_scalar_mul(attn_r[:], exp_sb[:], recip[:])
                    nc.sync.dma_start(
                        out=attnT[b, jb * 128:(jb + 1) * 128, :], in_=attn_r[:])
                    for (c0, cw) in ICH:
                        nc.tensor.matmul(po[:, c0:c0 + cw],
                                         v_sb[:, jb * Z:(jb + 1) * Z],
                                         attn_r[:, c0:c0 + cw],
                                         start=(jb == 0), stop=(jb == JB - 1))

                # ---- output ----
                ob = osb.tile([Z, npack], F32, tag="ob")
                nc.scalar.copy(ob[:], po[:])
                nc.sync.dma_start(out=outT[b, :, :], in_=ob[:])

            # software-pipelined batch loop: b+1's loads/splits are emitted at
            # the top of b's attention loop, its projection ~70% through, so
            # each engine's FIFO stream hits them with inputs already resident
            state = proj_v(0, load_split(0))
            for b in range(NB):
                hooks, carry = {}, {}
                if b + 1 < NB:
                    def _h0(bn=b + 1):
                        carry["ls"] = load_split(bn)
                    hooks = {1: _h0}
                attention(b, state, hooks)
                state = proj_v(b + 1, carry["ls"]) if b + 1 < NB else None

    nc.compile()
    return nc


def _get_program(npack):
    key = ("nc", npack)
    if key not in _CACHE:
        _CACHE[key] = build_program(npack)
    return _CACHE[key]


def kernel(encoder_hidden_states, decoder_hidden_states, decoder_final_hidden_state,
           latent_z_seq, mask, Wq, Wk, Wv):
    enc = np.asarray(encoder_hidden_states, dtype=np.float32)
    dec = np.asarray(decoder_hidden_states, dtype=np.float32)
    lat = np.asarray(latent_z_seq, dtype=np.float32)
    mask = np.asarray(mask)
    Wq = np.asarray(Wq, dtype=np.float32)
    Wk = np.asarray(Wk, dtype=np.float32)
    Wv = np.asarray(Wv, dtype=np.float32)

    # reparametrized logit weight, temperature folded (fp64 for exactness)
    w2 = ((Wq.astype(np.float64) / np.sqrt(H)) @ Wk.astype(np.float64).T
          ).astype(np.float32)

    # host prep: pack unmasked queries, transpose inputs
    counts = mask.sum(axis=1).astype(int)
    npack = int(-(-(counts.max() + 1) // 128) * 128)   # >=1 spare zero column
    decT = np.zeros((B, H, npack), dtype=np.float32)
    slots = np.empty((B, S), dtype=np.int64)
    for b in range(B):
        idx = np.flatnonzero(mask[b])
        n = len(idx)
        decT[b, :, :n] = dec[b, idx].T
        sl = np.full(S, n, dtype=np.int64)             # masked -> all-zero column
        sl[idx] = np.arange(n)
        slots[b] = sl
    encT = np.ascontiguousarray(enc.transpose(0, 2, 1))
    latT = np.ascontiguousarray(lat.transpose(0, 2, 1))

    nc = _get_program(npack)
    in_maps = []
    for c in range(NCORES):
        sl = slice(c * NB, (c + 1) * NB)
        in_maps.append({
            "decT": decT[sl], "encT": encT[sl], "latT": latT[sl],
            "w2": w2, "wv": Wv,
        })

    trace = bool(int(os.environ.get("ATTN_KERNEL_TRACE", "0")))
    if trace:
        import ntff_shim
        ntff_shim.install()
    res = run_bass_kernel_spmd(nc, in_maps, list(range(NCORES)), trace=trace)
    if trace:
        _CACHE["last_results"] = res

    attnT_all = np.concatenate([r["attnT"] for r in res.results], axis=0)
    outT_all = np.concatenate([r["outT"] for r in res.results], axis=0)

    # unpack: attn[b, i, j] = attnT[b, j, slot[i]] ; out[b, i, d] = outT[b, d, slot[i]]
    attn_g = np.empty((B, S, S), dtype=np.float32)   # [b, j, i]
    output = np.empty((B, S, Z), dtype=np.float32)
    for b in range(B):
        np.take(attnT_all[b], slots[b], axis=1, out=attn_g[b])
        output[b] = outT_all[b][:, slots[b]].T
    attn = attn_g.transpose(0, 2, 1)                 # view: [b, i, j]
    return output, attn


# revision 10
# speedup vs baseline: 1.2432x; 1.2432x over previous
"""Trainium2 Bass kernel for nn_Attention5 (sparse_attention).

Math (B=32, S=2048, H=256, Z=64):
    k = enc @ Wk ; q = dec @ Wq ; v = lat @ Wv
    A[b,i,j] = q_i . k_j / sqrt(H)
    A masked over QUERY axis i (rows with mask=0 -> -1e9)
    attn = softmax(A, axis=-2)          # over i (queries) for each key j
    out  = attn @ v                     # [B,S,Z]
    returns (out, attn)

Strategy:
  * Data-parallel over batch: 4 batches per core x 8 cores.
  * Reparametrize A = dec @ W2 @ enc.T with W2 = (Wq/sqrt(H)) @ Wk.T computed
    host-side in fp64 - removes the K projection entirely.
  * Device computes attn TRANSPOSED (attnT[j,i]) so the softmax reduction is
    along the free axis.  Host returns a transposed view.
  * Masked queries produce exactly-0 attn rows (exp underflow), identical to
    the reference.  Host packs the unmasked query columns into a fixed
    NPACK block (sized from the actual mask); zero columns are never computed
    or written (outputs are pre-zeroed), then host scatters back.
  * Precision: P = dec@W2 projection and P@enc.T logits use a 3-pass bf16
    split (hi*hi + hi*lo + lo*hi): ~5e-6 rel err at full bf16 matmul speed.
    (fp32 matmul measures 10x slower on TRN2; 1-pass f32r/tf32 is too coarse
    for the exp.)  attn.V runs in f32r (tf32).
"""
import sys, os
sys.path.insert(0, "/opt/trn_rl_repo")
import numpy as np

import concourse.bass as bass
import concourse.bacc as bacc
import concourse.tile as tile
import concourse.mybir as mybir
from concourse.bass_utils import run_bass_kernel_spmd

F32 = mybir.dt.float32
F32R = mybir.dt.float32r
BF16 = mybir.dt.bfloat16
AF = mybir.ActivationFunctionType
AX = mybir.AxisListType
OP = mybir.AluOpType

B, S, H, Z = 32, 2048, 256, 64
NCORES = 8
NB = B // NCORES          # batches per core
HB = H // 128             # 2 h-blocks (contraction)
GB = H // 128             # 2 g-blocks (projected dim of P)
JB = S // 128             # 16 key blocks

_CACHE = {}


def _chunks(width, step=512):
    out, c = [], 0
    while c < width:
        out.append((c, min(step, width - c)))
        c += step
    return out


def _split_pair(nc, src_ap, hi_tile, lo_tile, sub_engine):
    """hi = bf16(src); lo = bf16(src - hi). src may be SBUF or PSUM."""
    nc.scalar.copy(hi_tile[:], src_ap)
    sub_engine.tensor_tensor(lo_tile[:], src_ap, hi_tile[:], op=OP.subtract)


def build_program(npack):
    nc = bacc.Bacc("TRN2", target_bir_lowering=False, debug=False)
    ICH = _chunks(npack)

    decT = nc.dram_tensor("decT", [NB, H, npack], F32, kind="ExternalInput").ap()
    encT = nc.dram_tensor("encT", [NB, H, S], F32, kind="ExternalInput").ap()
    latT = nc.dram_tensor("latT", [NB, Z, S], F32, kind="ExternalInput").ap()
    w2 = nc.dram_tensor("w2", [H, H], F32, kind="ExternalInput").ap()
    wv = nc.dram_tensor("wv", [Z, Z], F32, kind="ExternalInput").ap()

    attnT = nc.dram_tensor("attnT", [NB, S, npack], F32R, kind="ExternalOutput").ap()
    outT = nc.dram_tensor("outT", [NB, Z, npack], F32, kind="ExternalOutput").ap()

    with tile.TileContext(nc) as tc:
        with (
            tc.tile_pool(name="wpool", bufs=1) as wpool,
            tc.tile_pool(name="io", bufs=1) as io,
            tc.tile_pool(name="split", bufs=2) as split,
            tc.tile_pool(name="qk", bufs=2) as qk,
            tc.tile_pool(name="vpool", bufs=2) as vpool,
            tc.tile_pool(name="epool", bufs=4) as epool,
            tc.tile_pool(name="small", bufs=6) as small,
            tc.tile_pool(name="osb", bufs=2) as osb,
            tc.tile_pool(name="atp", bufs=4, space="PSUM") as atp,
            tc.tile_pool(name="projp", bufs=1, space="PSUM") as projp,
            tc.tile_pool(name="outp", bufs=1, space="PSUM") as outp,
        ):
            # ---- weights: load + bf16 split (once) ----
            w2_b, w2_l = [], []
            for hb in range(HB):
                wf = wpool.tile([128, H], F32, tag=f"w2f{hb}", name=f"w2f{hb}")
                nc.sync.dma_start(out=wf[:], in_=w2[hb * 128:(hb + 1) * 128, :])
                hi = wpool.tile([128, H], BF16, tag=f"w2b{hb}", name=f"w2b{hb}")
                lo = wpool.tile([128, H], BF16, tag=f"w2l{hb}", name=f"w2l{hb}")
                _split_pair(nc, wf[:], hi, lo, nc.vector)
                w2_b.append(hi)
                w2_l.append(lo)
            wvf = wpool.tile([Z, Z], F32, tag="wvf")
            nc.sync.dma_start(out=wvf[:], in_=wv[:])
            wv_b = wpool.tile([Z, Z], BF16, tag="wvb")
            wv_l = wpool.tile([Z, Z], BF16, tag="wvl")
            _split_pair(nc, wvf[:], wv_b, wv_l, nc.vector)

            def load_split(b):
                # ---- load + split activations ----
                dec_b, dec_l, enc_b, enc_l = [], [], [], []
                for hb in range(HB):
                    df = io.tile([128, npack], F32, tag=f"df{hb}", name=f"df{hb}")
                    nc.sync.dma_start(out=df[:], in_=decT[b, hb * 128:(hb + 1) * 128, :])
                    dbt = split.tile([128, npack], BF16, tag=f"db{hb}", name=f"db{hb}")
                    dlt = split.tile([128, npack], BF16, tag=f"dl{hb}", name=f"dl{hb}")
                    _split_pair(nc, df[:], dbt, dlt, nc.gpsimd)
                    dec_b.append(dbt); dec_l.append(dlt)

                    ef = io.tile([128, S], F32, tag=f"ef{hb}", name=f"ef{hb}")
                    nc.sync.dma_start(out=ef[:], in_=encT[b, hb * 128:(hb + 1) * 128, :])
                    ebt = split.tile([128, S], BF16, tag=f"eb{hb}", name=f"eb{hb}")
                    elt = split.tile([128, S], BF16, tag=f"el{hb}", name=f"el{hb}")
                    _split_pair(nc, ef[:], ebt, elt, nc.gpsimd)
                    enc_b.append(ebt); enc_l.append(elt)

                lf = io.tile([Z, S], F32, tag="lf")
                nc.sync.dma_start(out=lf[:], in_=latT[b, :, :])
                lat_b = split.tile([Z, S], BF16, tag="latb")
                lat_l = split.tile([Z, S], BF16, tag="latl")
                _split_pair(nc, lf[:], lat_b, lat_l, nc.vector)

                return dec_b, dec_l, enc_b, enc_l, lat_b, lat_l

            def proj_v(b, ls_state):
                dec_b, dec_l, enc_b, enc_l, lat_b, lat_l = ls_state
                # ---- projection PT[g, i-pack] = W2.T @ decT (3-pass bf16) ----
                p_b, p_l = [], []
                for g in range(GB):
                    pb = qk.tile([128, npack], BF16, tag=f"pb{g}", name=f"pb{g}")
                    pl = qk.tile([128, npack], BF16, tag=f"pl{g}", name=f"pl{g}")
                    p_b.append(pb); p_l.append(pl)
                    for (c0, cw) in ICH:
                        ps = projp.tile([128, 512], F32, tag="pj", name="pproj2")
                        first = True
                        for hb in range(HB):
                            wsl_b = w2_b[hb][:, g * 128:(g + 1) * 128]
                            wsl_l = w2_l[hb][:, g * 128:(g + 1) * 128]
                            nc.tensor.matmul(ps[:, :cw], wsl_b,
                                             dec_b[hb][:, c0:c0 + cw],
                                             start=first, stop=False)
                            first = False
                            nc.tensor.matmul(ps[:, :cw], wsl_b,
                                             dec_l[hb][:, c0:c0 + cw],
                                             start=False, stop=False)
                            nc.tensor.matmul(ps[:, :cw], wsl_l,
                                             dec_b[hb][:, c0:c0 + cw],
                                             start=False, stop=(hb == HB - 1))
                        nc.scalar.copy(pb[:, c0:c0 + cw], ps[:, :cw])
                        nc.vector.tensor_tensor(pl[:, c0:c0 + cw], ps[:, :cw],
                                                pb[:, c0:c0 + cw], op=OP.subtract)

                # ---- V[j, d] (3-pass bf16), rounded to f32r ----
                v_sb = vpool.tile([128, JB * Z], F32R, tag="vsb")
                for jb in range(JB):
                    ps = projp.tile([128, 512], F32, tag="pj", name="pv")
                    lsl_b = lat_b[:, jb * 128:(jb + 1) * 128]
                    lsl_l = lat_l[:, jb * 128:(jb + 1) * 128]
                    nc.tensor.matmul(ps[:, :Z], lsl_b, wv_b[:], start=True, stop=False)
                    nc.tensor.matmul(ps[:, :Z], lsl_b, wv_l[:], start=False, stop=False)
                    nc.tensor.matmul(ps[:, :Z], lsl_l, wv_b[:], start=False, stop=True)
                    nc.scalar.copy(v_sb[:, jb * Z:(jb + 1) * Z], ps[:, :Z])

                return enc_b, enc_l, p_b, p_l, v_sb

            def attention(b, state, mid_hooks):
                enc_b, enc_l, p_b, p_l, v_sb = state
                # ---- attention over key blocks: A^T[j,i] = enc @ P^T ----
                po = outp.tile([Z, npack], F32, tag="po")
                for jb in range(JB):
                    if jb in mid_hooks:
                        mid_hooks[jb]()
                    chunks_ps = [atp.tile([128, 512], F32, tag="at", name=f"pat{ci}")
                                 for ci in range(len(ICH))]
                    # stationary-major: 6 stationaries x all chunks each
                    started = [False] * len(ICH)
                    n_mm = [0] * len(ICH)
                    passes = []
                    for hb in range(HB):
                        esl_b = enc_b[hb][:, jb * 128:(jb + 1) * 128]
                        esl_l = enc_l[hb][:, jb * 128:(jb + 1) * 128]
                        passes += [(esl_b, p_b[hb]), (esl_b, p_l[hb]),
                                   (esl_l, p_b[hb])]
                    for pi, (stat, mov) in enumerate(passes):
                        for ci, (c0, cw) in enumerate(ICH):
                            nc.tensor.matmul(chunks_ps[ci][:, :cw], stat,
                                             mov[:, c0:c0 + cw],
                                             start=(pi == 0),
                                             stop=(pi == len(passes) - 1))

                    # row max (negated) over the packed width
                    nmaxes = []
                    for ci, (c0, cw) in enumerate(ICH):
                        nm_ = small.tile([128, 1], F32, tag="nm", name=f"nm{ci}")
                        nc.vector.tensor_reduce(nm_[:], chunks_ps[ci][:, :cw],
                                                axis=AX.X, op=OP.max, negate=True)
                        nmaxes.append(nm_)
                    nmax = nmaxes[0]
                    for ci in range(1, len(ICH)):
                        nm2 = small.tile([128, 1], F32, tag="nmax", name=f"nmx{ci}")
                        nc.vector.tensor_tensor(nm2[:], nmax[:], nmaxes[ci][:],
                                                op=OP.min)
                        nmax = nm2

                    # exp(A - max) with per-chunk row-sum accumulation
                    exp_sb = epool.tile([128, npack], F32, tag="exp")
                    parts = []
                    for ci, (c0, cw) in enumerate(ICH):
                        pa = small.tile([128, 1], F32, tag="pa", name=f"pa{ci}")
                        nc.scalar.activation(exp_sb[:, c0:c0 + cw],
                                             chunks_ps[ci][:, :cw], AF.Exp,
                                             bias=nmax[:], scale=1.0,
                                             accum_out=pa[:])
                        parts.append(pa)
                    rsum = parts[0]
                    for ci in range(1, len(ICH)):
                        rs2 = small.tile([128, 1], F32, tag="rsum", name=f"rs{ci}")
                        nc.vector.tensor_tensor(rs2[:], rsum[:], parts[ci][:],
                                                op=OP.add)
                        rsum = rs2
                    recip = small.tile([128, 1], F32, tag="recip")
                    nc.vector.reciprocal(recip[:], rsum[:])

                    # normalize -> f32r attn rows; write out; accumulate output
                    attn_r = epool.tile([128, npack], F32R, tag="attnr")
                    nc.vector.tensor_scalar_mul(attn_r[:], exp_sb[:], recip[:])
                    nc.sync.dma_start(
                        out=attnT[b, jb * 128:(jb + 1) * 128, :], in_=attn_r[:])
                    for (c0, cw) in ICH:
                        nc.tensor.matmul(po[:, c0:c0 + cw],
                                         v_sb[:, jb * Z:(jb + 1) * Z],
                                         attn_r[:, c0:c0 + cw],
                                         start=(jb == 0), stop=(jb == JB - 1))

                # ---- output ----
                ob = osb.tile([Z, npack], F32, tag="ob")
                nc.scalar.copy(ob[:], po[:])
                nc.sync.dma_start(out=outT[b, :, :], in_=ob[:])

            # software-pipelined batch loop: b+1's loads/splits are emitted at
            # the top of b's attention loop, its projection ~70% through, so
            # each engine's FIFO stream hits them with inputs already resident
            for b in range(NB):
                attention(b, proj_v(b, load_split(b)), {})

    nc.compile()
    return nc


def _get_program(npack):
    key = ("nc", npack)
    if key not in _CACHE:
        _CACHE[key] = build_program(npack)
    return _CACHE[key]


def kernel(encoder_hidden_states, decoder_hidden_states, decoder_final_hidden_state,
           latent_z_seq, mask, Wq, Wk, Wv):
    enc = np.asarray(encoder_hidden_states, dtype=np.float32)
    dec = np.asarray(decoder_hidden_states, dtype=np.float32)
    lat = np.asarray(latent_z_seq, dtype=np.float32)
    mask = np.asarray(mask)
    Wq = np.asarray(Wq, dtype=np.float32)
    Wk = np.asarray(Wk, dtype=np.float32)
    Wv = np.asarray(Wv, dtype=np.float32)

    # reparametrized logit weight, temperature folded (fp64 for exactness)
    w2 = ((Wq.astype(np.float64) / np.sqrt(H)) @ Wk.astype(np.float64).T
          ).astype(np.float32)

    # host prep: pack unmasked queries, transpose inputs
    counts = mask.sum(axis=1).astype(int)
    npack = int(-(-(counts.max() + 1) // 128) * 128)   # >=1 spare zero column
    decT = np.zeros((B, H, npack), dtype=np.float32)
    slots = np.empty((B, S), dtype=np.int64)
    for b in range(B):
        idx = np.flatnonzero(mask[b])
        n = len(idx)
        decT[b, :, :n] = dec[b, idx].T
        sl = np.full(S, n, dtype=np.int64)             # masked -> all-zero column
        sl[idx] = np.arange(n)
        slots[b] = sl
    encT = np.ascontiguousarray(enc.transpose(0, 2, 1))
    latT = np.ascontiguousarray(lat.transpose(0, 2, 1))

    nc = _get_program(npack)
    in_maps = []
    for c in range(NCORES):
        sl = slice(c * NB, (c + 1) * NB)
        in_maps.append({
            "decT": decT[sl], "encT": encT[sl], "latT": latT[sl],
            "w2": w2, "wv": Wv,
        })

    trace = bool(int(os.environ.get("ATTN_KERNEL_TRACE", "0")))
    if trace:
        import ntff_shim
        ntff_shim.install()
    res = run_bass_kernel_spmd(nc, in_maps, list(range(NCORES)), trace=trace)
    if trace:
        _CACHE["last_results"] = res

    attnT_all = np.concatenate([r["attnT"] for r in res.results], axis=0)
    outT_all = np.concatenate([r["outT"] for r in res.results], axis=0)

    # unpack: attn[b, i, j] = attnT[b, j, slot[i]] ; out[b, i, d] = outT[b, d, slot[i]]
    attn_g = np.empty((B, S, S), dtype=np.float32)   # [b, j, i]
    output = np.empty((B, S, Z), dtype=np.float32)
    for b in range(B):
        np.take(attnT_all[b], slots[b], axis=1, out=attn_g[b])
        output[b] = outT_all[b][:, slots[b]].T
    attn = attn_g.transpose(0, 2, 1)                 # view: [b, i, j]
    return output, attn
